# revision 21
# baseline (speedup 1.0000x reference)
"""AttnBlock Bass/Tile kernel for TRN2 (v22: fp8 DoubleRow, merged drains).

Per-core computation (data-parallel over batch, BPC samples per core):
  h  = GroupNorm32(x) * gamma + beta            x: [C=512, T=1024] per sample
  u  = (16 Wk^T Wq) h                           fp8-DR GEMM, [ci, s]
  w' = u^T h  (scores, [s, t])                  fp8-DR
  we = exp(w' * SCALE/16 - 5 ln2 + r1)          fp8 e4m3 (max ~7 << 240)
  Z  = ones^T we  (replicated over partitions)  fp8-DR
  o2 = (vT^T we) * (1/Z)                        fp8-DR; vT = (16 Wv h)^T
  y  = x + (16 Wp^T)^T o2 / 256 + bp'           bp' = bp + Wp bv
All five big GEMMs run fp8 e4m3 with MatmulPerfMode.DoubleRow (K=256 per
matmul, 2 fp8 weights per PE cell) at ~2x the fp16 stream rate.  Host
pre-scales weights by 16 to center fp8 magnitudes (|u|<90,|v|<90,|we|<7,
|o2|<9 vs e4m3 max 240); the scales cancel via the exp scale, 1/Z, and a
1/256 epilogue factor.  exp output is pre-shifted 2^-5 via its bias
(cancels in 1/Z).  PSUM pair-tiles [128,1024] (2 banks) let every
PSUM->SBUF drain be a single [128,1024] op, halving DVE/Act instruction
count; exp/u-casts/vT-casts run on Act, GN/o2/1/Z/epilogue on DVE.
x loads ride 4 DMA queues; GroupNorm is split per channel-pair so h is
ready sooner; y is stored fp16 and the residual reuses the x tiles.
"""
import numpy as np
import ml_dtypes
from contextlib import ExitStack

import concourse.bass as bass
import concourse.tile as tile
from concourse import bacc, mybir

F32 = mybir.dt.float32
F16 = mybir.dt.float16
F8 = mybir.dt.float8e4
DRM = mybir.MatmulPerfMode.DoubleRow
AOT = mybir.AluOpType
AFT = mybir.ActivationFunctionType

C = 512
T = 1024
NCB = C // 128   # 4 channel blocks
NPR = C // 256   # 2 channel pairs (DoubleRow contraction super-blocks)
NSB = T // 128   # 8 key blocks
NSP = T // 256   # 4 key pairs
TT = 512         # t-tile (matmul moving free dim)
NTT = T // TT    # 2 t tiles
GROUPS = 32
CPG = C // GROUPS  # 16 channels per group
EPS = 1e-5
MS = 16.0        # host pre-scale on Wk^T Wq, Wv^T, Wp^T
ESH = 5.0        # exp output pre-shift: we *= 2^-ESH (cancels in 1/Z)
SCALE = float(C) ** -0.5
NV = 5           # packed per-cb vectors: gamma, beta, bp', bq, bv
N_WARM = 16
E4 = ml_dtypes.float8_e4m3


def aux_inputs(inputs=None):
    """Packed constants + host-precomputed fp8 weight tensors.

    consts [128, 130]: pmat | eps col | -ESH*ln2 col
    vecs   [128, NCB*NV]: per channel-block columns gamma,beta,bp',bq,bv
    mt8    [NPR, 128, 2, C] e4m3: 16 Wk^T Wq; [p][ki,ko,ci] = M[256p+128ko+ki, ci]
    wvt8   [NPR, 128, 2, C] e4m3: 16 Wv^T  (k = cin, free = cout)
    wpt8   [NPR, 128, 2, C] e4m3: 16 Wp^T
    ones8  [128, 2, 128] e4m3 ones (Z-sum stationary)
    rbk    [128, NCB] f16: Wk^T bq (key-side bias; all-zero for ref data)
    """
    consts = np.zeros((128, 130), dtype=np.float32)
    for c in range(128):
        for c2 in range(128):
            if c // CPG == c2 // CPG:
                consts[c, c2] = 1.0 / CPG
    consts[:, 128] = EPS
    consts[:, 129] = -ESH * np.log(2.0)

    def pair_layout(M):
        # M: [C(k), C(free)] -> [NPR, 128, 2, C] with k = 256p + 128ko + ki
        return np.ascontiguousarray(
            M.reshape(NPR, 2, 128, C).transpose(0, 2, 1, 3))

    out = {"consts": consts,
           "ones8": np.ones((128, 2, 128), dtype=E4)}
    if inputs is not None:
        Wq = np.asarray(inputs["Wq"], np.float64)
        Wk = np.asarray(inputs["Wk"], np.float64)
        Wv = np.asarray(inputs["Wv"], np.float64)
        Wp = np.asarray(inputs["Wp"], np.float64)
        bq = np.asarray(inputs["bq"], np.float64)
        bv = np.asarray(inputs["bv"], np.float64)
        bp = np.asarray(inputs["bp"], np.float64)
        bpp = (bp + Wp @ bv).astype(np.float32)
        vecs = np.zeros((128, NCB * NV), dtype=np.float32)
        cols = [np.asarray(inputs["gn_gamma"], np.float32),
                np.asarray(inputs["gn_beta"], np.float32),
                bpp,
                np.asarray(inputs["bq"], np.float32),
                np.asarray(inputs["bv"], np.float32)]
        for cb in range(NCB):
            for v in range(NV):
                vecs[:, cb * NV + v] = cols[v][cb * 128:(cb + 1) * 128]
        out["vecs"] = vecs
        out["mt8"] = pair_layout(np.clip(MS * (Wk.T @ Wq), -240, 240)).astype(E4)
        out["wvt8"] = pair_layout(np.clip(MS * Wv.T, -240, 240)).astype(E4)
        out["wpt8"] = pair_layout(np.clip(MS * Wp.T, -240, 240)).astype(E4)
        out["rbk"] = np.ascontiguousarray(
            (Wk.T @ bq).astype(np.float16).reshape(NCB, 128).T)
    else:
        out["vecs"] = np.zeros((128, NCB * NV), np.float32)
        out["mt8"] = np.zeros((NPR, 128, 2, C), E4)
        out["wvt8"] = np.zeros((NPR, 128, 2, C), E4)
        out["wpt8"] = np.zeros((NPR, 128, 2, C), E4)
        out["rbk"] = np.zeros((128, NCB), np.float16)
    return out


def build_nc(bpc=2, has_r1=False):
    nc = bacc.Bacc("TRN2", target_bir_lowering=False, debug=False,
                   enable_asserts=False)

    x_d = nc.dram_tensor("x", [bpc, C, T], F32, kind="ExternalInput")
    mt_d = nc.dram_tensor("mt8", [NPR, 128, 2, C], F8, kind="ExternalInput")
    wvt_d = nc.dram_tensor("wvt8", [NPR, 128, 2, C], F8, kind="ExternalInput")
    wpt_d = nc.dram_tensor("wpt8", [NPR, 128, 2, C], F8, kind="ExternalInput")
    ones_d = nc.dram_tensor("ones8", [128, 2, 128], F8, kind="ExternalInput")
    rbk_d = nc.dram_tensor("rbk", [128, NCB], F16, kind="ExternalInput")
    consts_d = nc.dram_tensor("consts", [128, 130], F32, kind="ExternalInput")
    vecs_d = nc.dram_tensor("vecs", [128, NCB * NV], F32, kind="ExternalInput")
    y_d = nc.dram_tensor("y", [bpc, C, T], F16, kind="ExternalOutput")

    with tile.TileContext(nc) as tc, ExitStack() as ctx:
        P = lambda **kw: ctx.enter_context(tc.tile_pool(**kw))
        singles = P(name="singles", bufs=1)
        wtp = P(name="wtp", bufs=1)        # mt8/wvt8/wpt8, persistent
        xp = P(name="xp", bufs=bpc * NCB)  # x blocks [128,1024] f32, live to epilogue
        hp = P(name="hp", bufs=4)          # h8 pair tiles [128,2,1024]
        up = P(name="up", bufs=4)          # u8 pair tiles [128,2,1024]
        vp = P(name="vp", bufs=8)          # vT8 pair tiles [128,2,512]
        wep = P(name="wep", bufs=8)        # we8 pair tiles [128,2,1024]
        o2p = P(name="o2p", bufs=4)        # o28 pair tiles [128,2,1024]
        rzp = P(name="rzp", bufs=2)        # 1/Z replicated [128,1024] f32
        yp = P(name="yp", bufs=4)          # y out tiles [128,1024] fp16
        smp = P(name="smp", bufs=8)        # small sbuf tiles
        r1p = P(name="r1p", bufs=16)
        ps_mm = P(name="ps_mm", bufs=3, space="PSUM")   # [128,1024] 2-bank tiles
        ps_wz = P(name="ps_wz", bufs=1, space="PSUM")   # warm / lead GN bc / Z

        # ---- PE warm-up: dummy matmuls on a zero tile while DMAs land ----
        wtile = singles.tile([128, TT], F16, tag="wtile", name="wtile")
        nc.vector.memset(wtile[:], 0.0)

        def warm(k):
            for _ in range(k):
                wps = ps_wz.tile([128, T], F32, tag="wz", name="warm_ps")
                nc.tensor.matmul(wps[:, 0:TT], wtile[:, 0:128], wtile[:],
                                 start=True, stop=True)

        warm(N_WARM)

        # ---- DMAs: consts tiny first; x spread over 4 queues; weights next
        consts = singles.tile([128, 130], F32, tag="consts", name="consts")
        nc.sync.dma_start(consts[:], consts_d.ap())
        vecs = singles.tile([128, NCB * NV], F32, tag="vecs", name="vecs")
        nc.sync.dma_start(vecs[:], vecs_d.ap())

        x_t = [None] * bpc
        h8_t = [None] * bpc

        def load_x(bb, engines=None):
            # two row-contiguous DMAs per block: more ring entries in flight
            x_t[bb] = []
            for cb in range(NCB):
                xt = xp.tile([128, T], F32, tag="x", name="x")
                eng = nc.gpsimd if engines is None else engines[cb]
                for hp_ in range(2):
                    eng.dma_start(
                        xt[hp_ * 64:(hp_ + 1) * 64, :],
                        x_d.ap()[bb, cb * 128 + hp_ * 64:cb * 128 + (hp_ + 1) * 64, :])
                x_t[bb].append(xt)

        ones8 = singles.tile([128, 2, 128], F8, tag="ones8", name="ones8")
        Mt8 = [wtp.tile([128, 2, C], F8, tag=f"mt{p}", name=f"mt{p}")
               for p in range(NPR)]
        WTv8 = [wtp.tile([128, 2, C], F8, tag=f"wv{p}", name=f"wv{p}")
                for p in range(NPR)]
        WTp8 = [wtp.tile([128, 2, C], F8, tag=f"wp{p}", name=f"wp{p}")
                for p in range(NPR)]
        # queue priority: gpsimd: WTv0, x1, WTv1 | sync: x0, x3, Mt | scalar:
        # ones8, x2, WTp (needed last).  GN pair0 gates on x0/x1, pair1 x2/x3.
        # x: one block per queue; 4th block split row-wise across gpsimd+
        # scalar so no queue carries two full blocks.  Weights follow x.
        x_t[0] = []
        for cb in range(NCB):
            x_t[0].append(xp.tile([128, T], F32, tag="x", name="x"))
        xa = x_d.ap()
        nc.sync.dma_start(x_t[0][0][:], xa[0, 0:128, :])
        nc.gpsimd.dma_start(x_t[0][1][:], xa[0, 128:256, :])
        nc.scalar.dma_start(x_t[0][2][:], xa[0, 256:384, :])
        nc.gpsimd.dma_start(x_t[0][3][0:64, :], xa[0, 384:448, :])
        nc.scalar.dma_start(x_t[0][3][64:128, :], xa[0, 448:512, :])
        for p in range(NPR):
            nc.sync.dma_start(Mt8[p][:], mt_d.ap()[p])
        nc.gpsimd.dma_start(WTv8[0][:], wvt_d.ap()[0])
        nc.gpsimd.dma_start(WTv8[1][:], wvt_d.ap()[1])
        nc.scalar.dma_start(ones8[:], ones_d.ap())
        for p in range(NPR):
            nc.scalar.dma_start(WTp8[p][:], wpt_d.ap()[p])

        pmat = consts[:, 0:128]
        eps_t = consts[:, 128:129]
        ebias_t = consts[:, 129:130]       # -ESH*ln2 (exp bias, r1-free path)

        if has_r1:
            rbk_sb = singles.tile([128, NCB], F16, tag="rbk", name="rbk")
            nc.sync.dma_start(rbk_sb[:], rbk_d.ap())

        def group_norm(bb, n_act_writes=2):
            # per channel-pair so h lands incrementally; h-writes split DVE/Act
            # (prefetched samples keep Act free for the scores-loop exps)
            h8_t[bb] = [hp.tile([128, 2, T], F8, tag="h", name="h")
                        for _ in range(NPR)]
            for p in range(NPR):
                mv = smp.tile([128, 4], F32, tag="mv", name="mv")
                for i, cb in enumerate((2 * p, 2 * p + 1)):
                    stats = smp.tile([128, 2, 6], F32, tag="bnstats",
                                     name="bnstats")
                    for half in range(2):
                        nc.vector.bn_stats(
                            stats[:, half, :],
                            x_t[bb][cb][:, half * 512:(half + 1) * 512])
                    nc.vector.bn_aggr(mv[:, i:i + 3:2], stats[:])
                t2 = smp.tile([128, 2], F32, tag="t2", name="t2")
                nc.vector.tensor_mul(t2[:], mv[:, 0:2], mv[:, 0:2])
                nc.vector.tensor_add(mv[:, 2:4], mv[:, 2:4], t2[:])
                if bb == 0:
                    bc = ps_wz.tile([128, 4], F32, tag="wz", name="bc")
                else:
                    bc = ps_mm.tile([128, 4], F32, tag="mm", name="bc")
                nc.tensor.matmul(bc[:], pmat[:], mv[:], start=True, stop=True)
                chs = smp.tile([128, 4], F32, tag="chs", name="chs")
                nc.vector.tensor_copy(chs[:], bc[:])
                nc.vector.tensor_mul(t2[:], chs[:, 0:2], chs[:, 0:2])
                nc.vector.tensor_sub(chs[:, 2:4], chs[:, 2:4], t2[:])
                nc.scalar.activation(out=chs[:, 2:4], in_=chs[:, 2:4],
                                     func=AFT.Sqrt, bias=eps_t[:], scale=1.0)
                nc.vector.reciprocal(chs[:, 2:4], chs[:, 2:4])
                AB = smp.tile([128, 4], F32, tag="AB", name="AB")
                nc.vector.tensor_mul(AB[:, 0:2], chs[:, 2:4],
                                     vecs[:, 10 * p:10 * p + 6:5])
                nc.vector.tensor_mul(AB[:, 2:4], chs[:, 0:2], AB[:, 0:2])
                nc.vector.tensor_sub(AB[:, 2:4],
                                     vecs[:, 10 * p + 1:10 * p + 7:5],
                                     AB[:, 2:4])
                for i, cb in enumerate((2 * p, 2 * p + 1)):
                    if i == 0 or n_act_writes == 0:
                        nc.vector.tensor_scalar(
                            out=h8_t[bb][p][:, i, :], in0=x_t[bb][cb][:],
                            scalar1=AB[:, i:i + 1], scalar2=AB[:, i + 2:i + 3],
                            op0=AOT.mult, op1=AOT.add)
                    else:
                        nc.scalar.activation(
                            out=h8_t[bb][p][:, i, :], in_=x_t[bb][cb][:],
                            func=AFT.Identity, bias=AB[:, i + 2:i + 3],
                            scale=AB[:, i:i + 1])

        group_norm(0)


        # ---- per-sample attention stages (all GEMMs fp8 DoubleRow) ----
        u8_t = [None] * bpc

        def compute_u(bb):
            # u = (16 M) h : [ci, s]
            h8 = h8_t[bb]
            u8_t[bb] = [up.tile([128, 2, T], F8, tag="u", name="u")
                        for _ in range(NPR)]
            for cib in range(NCB):
                pu = ps_mm.tile([128, T], F32, tag="mm", name="u_ps")
                for p in range(NPR):
                    for st in range(NTT):
                        nc.tensor.matmul(
                            pu[:, st * TT:(st + 1) * TT],
                            Mt8[p][:, :, cib * 128:(cib + 1) * 128],
                            h8[p][:, :, st * TT:(st + 1) * TT],
                            start=(p == 0), stop=(p == NPR - 1), perf_mode=DRM)
                if cib % 2 == 0:
                    nc.scalar.copy(u8_t[bb][cib // 2][:, cib % 2, :], pu[:])
                else:
                    nc.vector.tensor_copy(u8_t[bb][cib // 2][:, cib % 2, :],
                                          pu[:])

        def sample(bb, prefetch_next):
            h8 = h8_t[bb]
            u8 = u8_t[bb]

            # r1[s] = (Wk^T bq) . h_s (skipped when bq == 0)
            r1_t = []
            if has_r1:
                for sb in range(NSB):
                    psr = ps_wz.tile([128, 1], F32, tag="wz", name="psr")
                    for cjb in range(NCB):
                        nc.tensor.matmul(
                            psr[:],
                            h8[cjb // 2][:, cjb % 2, sb * 128:(sb + 1) * 128],
                            rbk_sb[:, cjb:cjb + 1], start=(cjb == 0),
                            stop=(cjb == NCB - 1))
                    r1 = r1p.tile([128, 1], F32, tag="r1", name="r1")
                    nc.vector.tensor_scalar(out=r1[:], in0=psr[:],
                                            scalar1=SCALE,
                                            scalar2=-ESH * float(np.log(2.0)),
                                            op0=AOT.mult, op1=AOT.add)
                    r1_t.append(r1)

            if prefetch_next is not None:
                load_x(prefetch_next)
                group_norm(prefetch_next, n_act_writes=0)

            # scores + exp, vT and Z matmuls interleaved per key-pair so the
            # PE stays fed while exp (Act) drains each scores PSUM tile
            vt8 = [vp.tile([128, 2, C], F8, tag="vT", name="vT")
                   for _ in range(NSP)]
            we8 = [wep.tile([128, 2, T], F8, tag="we", name="we")
                   for _ in range(NSP)]
            rz = rzp.tile([128, T], F32, tag="rz", name="rz")
            zt = ps_wz.tile([128, T], F32, tag="wz", name="zps")

            def scores_sb(sb):
                pw = ps_mm.tile([128, T], F32, tag="mm", name="sc_ps")
                for p in range(NPR):
                    for tt in range(NTT):
                        nc.tensor.matmul(
                            pw[:, tt * TT:(tt + 1) * TT],
                            u8[p][:, :, sb * 128:(sb + 1) * 128],
                            h8[p][:, :, tt * TT:(tt + 1) * TT],
                            start=(p == 0), stop=(p == NPR - 1), perf_mode=DRM)
                bias = r1_t[sb][:] if has_r1 else ebias_t[:]
                nc.scalar.activation(
                    out=we8[sb // 2][:, sb % 2, :], in_=pw[:],
                    func=AFT.Exp, bias=bias, scale=SCALE / MS)

            for sp in range(NSP):
                scores_sb(2 * sp)
                pv = ps_mm.tile([128, T], F32, tag="mm", name="v_ps")
                for half in range(2):
                    sb = 2 * sp + half
                    for p in range(NPR):
                        nc.tensor.matmul(
                            pv[:, half * C:(half + 1) * C],
                            h8[p][:, :, sb * 128:(sb + 1) * 128],
                            WTv8[p][:, :, :],
                            start=(p == 0), stop=(p == NPR - 1), perf_mode=DRM)
                scores_sb(2 * sp + 1)
                nc.vector.tensor_copy(vt8[sp][:, :, :], pv[:])
                for tt in range(NTT):
                    nc.tensor.matmul(
                        zt[:, tt * TT:(tt + 1) * TT], ones8[:, :, :],
                        we8[sp][:, :, tt * TT:(tt + 1) * TT],
                        start=(sp == 0), stop=(sp == NSP - 1), perf_mode=DRM)
            if prefetch_next is not None:
                compute_u(prefetch_next)
            nc.vector.reciprocal_approx_fast(out=rz[:], in_=zt[:])
            # attention @ v, normalized by 1/Z on the way to SBUF (fp8 out)
            o28 = [o2p.tile([128, 2, T], F8, tag="o2", name="o2")
                   for _ in range(NPR)]
            for cb in range(NCB):
                pa = ps_mm.tile([128, T], F32, tag="mm", name="at_ps")
                for p in range(NSP):
                    for tt in range(NTT):
                        nc.tensor.matmul(
                            pa[:, tt * TT:(tt + 1) * TT],
                            vt8[p][:, :, cb * 128:(cb + 1) * 128],
                            we8[p][:, :, tt * TT:(tt + 1) * TT],
                            start=(p == 0), stop=(p == NSP - 1), perf_mode=DRM)
                nc.vector.tensor_tensor(
                    out=o28[cb // 2][:, cb % 2, :], in0=pa[:], in1=rz[:],
                    op=AOT.mult)

            # proj + epilogue: y = x + (256 Wp o2_true)/256 + bp'
            for cob in range(NCB):
                pj = ps_mm.tile([128, T], F32, tag="mm", name="pj_ps")
                for p in range(NPR):
                    for tt in range(NTT):
                        nc.tensor.matmul(
                            pj[:, tt * TT:(tt + 1) * TT],
                            WTp8[p][:, :, cob * 128:(cob + 1) * 128],
                            o28[p][:, :, tt * TT:(tt + 1) * TT],
                            start=(p == 0), stop=(p == NPR - 1), perf_mode=DRM)
                yt = yp.tile([128, T], F16, tag="y", name="y")
                nc.vector.scalar_tensor_tensor(
                    out=yt[:], in0=pj[:], scalar=1.0 / (MS * MS),
                    in1=x_t[bb][cob][:], op0=AOT.mult, op1=AOT.add)
                nc.sync.dma_start(
                    y_d.ap()[bb, cob * 128:(cob + 1) * 128, :], yt[:])

        compute_u(0)
        for bb in range(bpc):
            sample(bb, bb + 1 if bb + 1 < bpc else None)

    nc.compile()
    return nc


# ---------------------------------------------------------------------------
# Harness entry point: full (unsharded) inputs -> full output.
# Shards batch 16 -> 2 samples on each of 8 NeuronCores (pure data parallel).
# ---------------------------------------------------------------------------
from concourse.bass_utils import run_bass_kernel_spmd

N_CORES = 8
_NC_CACHE = {}


def _get_nc(bpc, has_r1=False):
    key = (bpc, has_r1)
    if key not in _NC_CACHE:
        _NC_CACHE[key] = build_nc(bpc=bpc, has_r1=has_r1)
    return _NC_CACHE[key]


def kernel(x, gn_gamma, gn_beta, Wq, bq, Wk, bk, Wv, bv, Wp, bp):
    x = np.ascontiguousarray(np.asarray(x, dtype=np.float32))
    B = x.shape[0]
    assert B % N_CORES == 0, (B, N_CORES)
    bpc = B // N_CORES
    xr = x.reshape(B, C, T)
    aux = aux_inputs({"gn_gamma": gn_gamma, "gn_beta": gn_beta,
                      "bp": bp, "bq": bq, "bv": bv,
                      "Wq": Wq, "Wk": Wk, "Wv": Wv, "Wp": Wp})
    has_r1 = bool(np.any(aux["rbk"]))
    in_maps = []
    for c in range(N_CORES):
        m = {"x": np.ascontiguousarray(xr[c * bpc:(c + 1) * bpc])}
        m.update(aux)
        in_maps.append(m)
    nc = _get_nc(bpc, has_r1)
    res = run_bass_kernel_spmd(nc, in_maps, core_ids=list(range(N_CORES)))
    y = np.concatenate([np.asarray(res.results[c]["y"], np.float32)
                        .reshape(bpc, C, 32, 32)
                        for c in range(N_CORES)], axis=0)
    return y
